# revision 34
# baseline (speedup 1.0000x reference)
"""Multi-head causal attention with RoPE on 8 Trainium2 NeuronCores.

Reference computation (B=2, T=2048, C=1024, H=16, Dh=64, fp32):
    qkv = x @ w_qkv + b_qkv ; split q,k,v ; RoPE(q), RoPE(k)
    attn = softmax_causal(q k^T / sqrt(Dh)) @ v ; out = attn @ w_proj + b_proj

Sharding: core c = b*4 + g handles batch b and head group g (heads 4g..4g+3).
Data-parallel over batch, tensor-parallel over heads (w_qkv column-split,
w_proj row-split).  Each core emits a partial [T, C] projection output in
bf16 (halves the write drain); the host sums the 4 per-batch partials in
fp32 and adds b_proj.

Per-core kernel, organized to keep the PE tensor engine saturated end to end
(the HAM clock gate re-throttles the PE to 1.2 GHz after any idle window, so
PE gaps are doubly expensive):
  - DMA: x^T chunks stream on the sync+scalar queues while weights lead the
    gpsimd queue; rope tables (bf16) and w_proj ride the vector queue.  The
    first QKV matmuls start as soon as chunk 0 lands.
  - Phase 1a computes Q^T/K^T for heads 0-1 CHUNK-major across 8 PSUM banks
    so the PE chases the x DMA instead of stalling on the full tensor.
    RoPE fuses bias into the accumulation (rank-1 matmul), then
    qkt = pq*cos + perm @ (pq*sin_perm), the permutation matmul reusing the
    same PSUM bank in place.
  - Phase 1b computes V in [token, head*65] layout (65th col = ones so the
    PV matmul also emits the softmax denominator).
  - Phase 2 streams attention per head, span by span, in units of TWO
    k-tiles sharing a [128, 1024] two-bank PSUM group: one exp ACTIVATE
    covers both tiles (the ACT engine has a 352-cycle fixed cost per
    instruction, so batching matters), with the 1/sqrt(Dh) scale fused and
    causality via per-tile q-range narrowing plus a triangular mask matmul
    on diagonal tiles.  S^T tiles (scores transposed) make the softmax sum
    direction match the PE contraction.  A 2-unit lookahead queue keeps the
    PE 2 units ahead of the ACT exp.
  - Softmax normalization has NO ACT work (the baseline's exp(-ln(d)) tables
    thrashed ACT_TABLE_LOAD): the denominator row is broadcast across
    partitions with a rank-1 fp32r matmul, inverted with the DVE
    reciprocal_approx_fast op (~18 bits, plenty for bf16 storage), and
    multiplied in on the DVE.
  - Q/K for heads 2-3 are computed BETWEEN the head 0-1 and head 2-3
    attention streams (kc-inner, one flex PSUM bank), and the output
    projection for spans 0-2 is interleaved INTO head 3's stream so the
    [T, C] fp32 result DMA overlaps compute instead of trailing it.
All heavy matmuls run in bf16 (fp32 accumulation in PSUM); end-to-end rel
err ~5e-3 of output absmax.
"""

import numpy as np
import ml_dtypes

import concourse.bacc as bacc
import concourse.bass as bass
import concourse.mybir as mybir
from concourse.tile import TileContext
from concourse.bass_utils import run_bass_kernel_spmd

F32 = mybir.dt.float32
F32R = mybir.dt.float32r
BF16 = mybir.dt.bfloat16
NPBF16 = np.dtype(ml_dtypes.bfloat16)

B, T, C = 2, 2048, 1024
H, DH = 16, 64
GH = 4  # heads per core
N_CORES = 8
NCHUNK = C // 128  # 8 contraction chunks
NT = T // 128  # 16 token tiles
NSPAN = T // 512  # 4 query spans
QK_COLS = 2 * GH * DH  # 512 = q cols (256) + k cols (256)
VA = GH * (DH + 1)  # 260 = v cols augmented with ones column per head
EXP = mybir.ActivationFunctionType.Exp


def _build():
    nc = bacc.Bacc("TRN2", target_bir_lowering=False, debug=False, num_devices=N_CORES)

    xT = nc.dram_tensor("xT", [C, T], BF16, kind="ExternalInput")
    wqk = nc.dram_tensor("wqk", [C, QK_COLS], BF16, kind="ExternalInput")
    wv = nc.dram_tensor("wv", [C, VA], BF16, kind="ExternalInput")
    bqk_d = nc.dram_tensor("bqk", [1, QK_COLS], BF16, kind="ExternalInput")
    bv_d = nc.dram_tensor("bv", [1, VA], BF16, kind="ExternalInput")
    cos_d = nc.dram_tensor("cosT", [128, T], BF16, kind="ExternalInput")
    sinp_d = nc.dram_tensor("sinTp", [128, T], BF16, kind="ExternalInput")
    perm_d = nc.dram_tensor("perm", [128, 128], BF16, kind="ExternalInput")
    maskT_d = nc.dram_tensor("maskT", [128, 128], BF16, kind="ExternalInput")
    id_d = nc.dram_tensor("id128", [128, 128], BF16, kind="ExternalInput")
    wproj_d = nc.dram_tensor("wproj", [2, 128, C], BF16, kind="ExternalInput")
    # bf16 output halves the 8 MiB write drain; the host sums the four
    # per-batch partials in fp32.
    out_d = nc.dram_tensor("out", [T, C], BF16, kind="ExternalOutput")

    with TileContext(nc) as tc:
        with tc.tile_pool(name="persist", bufs=1) as pers:
            # x^T chunks lead the two hardware DMA queues (they gate the
            # phase-1a chase); wqk leads the gpsimd software queue (its ~12us
            # startup latency roughly matches the first x chunks' arrival).
            xt = []
            for kc in range(NCHUNK):
                t = pers.tile([128, T], BF16, tag="xt", bufs=NCHUNK, name=f"xt{kc}")
                eng = nc.scalar if kc % 2 == 0 else nc.sync
                eng.dma_start(out=t, in_=xT[128 * kc : 128 * (kc + 1), :])
                xt.append(t)
            wqk_t = []
            for kc in range(NCHUNK):
                t = pers.tile(
                    [128, QK_COLS], BF16, tag="wqk", bufs=NCHUNK, name=f"wqk{kc}"
                )
                nc.gpsimd.dma_start(out=t, in_=wqk[128 * kc : 128 * (kc + 1), :])
                wqk_t.append(t)
            cos_sb = pers.tile([128, T], BF16, tag="cos")
            nc.gpsimd.dma_start(out=cos_sb, in_=cos_d[:, :])
            sinp_sb = pers.tile([128, T], BF16, tag="sinp")
            nc.gpsimd.dma_start(out=sinp_sb, in_=sinp_d[:, :])
            wv_t = []
            for kc in range(NCHUNK):
                t = pers.tile([128, VA], BF16, tag="wv", bufs=NCHUNK, name=f"wv{kc}")
                nc.gpsimd.dma_start(out=t, in_=wv[128 * kc : 128 * (kc + 1), :])
                wv_t.append(t)
            bqk_sb = pers.tile([1, QK_COLS], BF16, tag="bqk")
            nc.gpsimd.dma_start(out=bqk_sb, in_=bqk_d[:, :])
            bv_sb = pers.tile([1, VA], BF16, tag="bv")
            nc.gpsimd.dma_start(out=bv_sb, in_=bv_d[:, :])
            perm_sb = pers.tile([128, 128], BF16, tag="perm")
            nc.gpsimd.dma_start(out=perm_sb, in_=perm_d[:, :])
            mask_sb = pers.tile([128, 128], BF16, tag="maskT")
            nc.gpsimd.dma_start(out=mask_sb, in_=maskT_d[:, :])
            id_sb = pers.tile([128, 128], BF16, tag="id128")
            nc.gpsimd.dma_start(out=id_sb, in_=id_d[:, :])
            wproj_sb = []
            for p in range(2):
                t = pers.tile([128, C], BF16, tag="wproj", bufs=2, name=f"wproj{p}")
                nc.gpsimd.dma_start(out=t, in_=wproj_d[p, :, :])
                wproj_sb.append(t)

            ones = pers.tile([1, 512], BF16, tag="ones")
            nc.vector.memset(ones, 1.0)
            ones_ff = pers.tile([128, 64], F32, tag="ones_ff")
            nc.vector.memset(ones_ff, 1.0)
            ones_r = pers.tile([128, 64], F32R, tag="ones_r")
            nc.vector.tensor_copy(ones_r, ones_ff)

            # Outputs of phase 1 (live into phase 2/3)
            qkt = []  # 4 tiles [128, T]: Q heads(0,1), Q(2,3), K(0,1), K(2,3)
            for i in range(4):
                t = pers.tile([128, T], BF16, tag="qkt", bufs=4, name=f"qkt{i}")
                qkt.append(t)
            vaug = []  # 16 tiles [128, VA], k-tile-major natural layout V
            for j in range(NT):
                t = pers.tile([128, VA], BF16, tag="vaug", bufs=NT, name=f"vaug{j}")
                vaug.append(t)
            attn = []  # 2 tiles [128, T]: normalized attn^T for head pairs
            for p in range(2):
                t = pers.tile([128, T], BF16, tag="attn", bufs=2, name=f"attn{p}")
                attn.append(t)

            # ------------- Phase 1a: Q/K heads 0-1, chunk-major -------------
            # One PSUM bank per (ct, span); all 8 banks live so each arriving
            # x chunk feeds 8 back-to-back matmuls.  ct=2 (K) first so the
            # attention stream's S matmuls unblock earliest.  After the bias,
            # each combo is staged to SBUF bf16 by the idle ACT engine: this
            # frees the PSUM banks immediately so V starts right away, and
            # the rope finish runs as bf16 DVE work overlapped with V.
            combosA = [(ct, sp) for ct in (2, 0) for sp in range(NSPAN)]
            with tc.tile_pool(name="p1a", bufs=1) as p1a:
                pqsb = []
                with tc.tile_pool(name="p1aps", bufs=1, space="PSUM") as p1aps:
                    psqk = [
                        p1aps.tile([128, 512], F32, tag="psqk", bufs=8, name=f"psqk{i}")
                        for i in range(8)
                    ]
                    for kc in range(NCHUNK):
                        for i, (ct, sp) in enumerate(combosA):
                            cs = slice(128 * ct, 128 * (ct + 1))
                            ss = slice(512 * sp, 512 * (sp + 1))
                            nc.tensor.matmul(
                                psqk[i],
                                wqk_t[kc][:, cs],
                                xt[kc][:, ss],
                                start=(kc == 0),
                                stop=False,
                            )
                    # Staging order matters: phase 1b's psv tiles reuse the
                    # banks of the first-staged combos (V chases copies 0-5 in
                    # order) while the rope's ppr tiles land on the last two
                    # banks — so combos 6,7 are staged right after 0,1 or the
                    # first perm matmul waits for the final copy.
                    pqsb = [None] * 8
                    for i in (0, 1, 6, 7, 2, 3, 4, 5):
                        ct, sp = combosA[i]
                        cs = slice(128 * ct, 128 * (ct + 1))
                        nc.tensor.matmul(
                            psqk[i], bqk_sb[0:1, cs], ones, start=False, stop=True
                        )
                        t = p1a.tile([128, 512], BF16, tag="pqsb", bufs=8, name="pqsb")
                        # All staging copies on ACT: psv bufs=6 gives the V
                        # chain enough slack that queueing behind these is
                        # fine, and the DVE stays clear for the rope chain.
                        nc.scalar.copy(t, psqk[i])
                        pqsb[i] = t

                # ------------- Phase 1b: V + rope finish --------------------
                # V's vaug copies ride ACT; the rope combos (2 bf16 DVE muls,
                # one perm matmul, one DVE add) are injected between V tiles
                # so the DVE rope chain hides entirely under V's PE work.
                with tc.tile_pool(name="p1bps", bufs=1, space="PSUM") as p1bps:
                    def rope_finish(i):
                        ct, sp = combosA[i]
                        ss = slice(512 * sp, 512 * (sp + 1))
                        t2 = p1a.tile([128, 512], BF16, tag="t2", bufs=2, name="t2")
                        nc.vector.tensor_mul(t2, pqsb[i], sinp_sb[:, ss])
                        qc = p1a.tile([128, 512], BF16, tag="qc", bufs=2, name="qc")
                        nc.vector.tensor_mul(qc, pqsb[i], cos_sb[:, ss])
                        pp = p1bps.tile(
                            [128, 512], F32, tag="ppr", bufs=2, name="ppr"
                        )
                        nc.tensor.matmul(pp, perm_sb, t2, start=True, stop=True)
                        nc.vector.tensor_add(qkt[ct][:, ss], qc, pp)

                    for it in range(NT):
                        pv = p1bps.tile([128, VA], F32, tag="psv", bufs=6, name="psv")
                        ts = slice(128 * it, 128 * (it + 1))
                        for kc in range(NCHUNK):
                            nc.tensor.matmul(
                                pv, xt[kc][:, ts], wv_t[kc],
                                start=(kc == 0), stop=False,
                            )
                        nc.tensor.matmul(
                            pv, ones[0:1, 0:128], bv_sb, start=False, stop=True
                        )
                        nc.scalar.copy(vaug[it], pv)
                        # Tiles 0-3 run clean while ACT finishes the pqsb
                        # staging; the rope chain starts at tile 4 with its
                        # inputs already in SBUF.
                        if 4 <= it < 12:
                            rope_finish(it - 4)

            # ------------- Phase 2: attention + interleaved extras ----------
            # Q/K for heads 2-3 and the output projection run as PE filler
            # INSIDE the attention streams: the per-unit exp on ACT (1.15us)
            # outpaces the unit's own matmuls (0.85us), and any sustained PE
            # idle re-throttles the HAM clock gate to 1.2 GHz.
            with (
                tc.tile_pool(name="p2", bufs=1) as p2,
                tc.tile_pool(name="p2ps", bufs=1, space="PSUM") as p2ps,
            ):

                def qkb_pieces(ct, sp):
                    # Q/K heads 2-3, one span column: split into 3 filler
                    # pieces (the flex PSUM bank is held across all three).
                    cs = slice(128 * ct, 128 * (ct + 1))
                    ss = slice(512 * sp, 512 * (sp + 1))
                    box = {}

                    def pa():
                        pq = p2ps.tile(
                            [128, 512], F32, tag="flex", bufs=1, name="pqB"
                        )
                        box["pq"] = pq
                        for kc in range(4):
                            nc.tensor.matmul(
                                pq, wqk_t[kc][:, cs], xt[kc][:, ss],
                                start=(kc == 0), stop=False,
                            )

                    def pb():
                        pq = box["pq"]
                        for kc in range(4, NCHUNK):
                            nc.tensor.matmul(
                                pq, wqk_t[kc][:, cs], xt[kc][:, ss],
                                start=False, stop=False,
                            )
                        nc.tensor.matmul(
                            pq, bqk_sb[0:1, cs], ones, start=False, stop=True
                        )

                    def pc():
                        pq = box["pq"]
                        t2 = p2.tile([128, 512], BF16, tag="t2b", bufs=2, name="t2b")
                        nc.vector.tensor_mul(t2, pq, sinp_sb[:, ss])
                        qc = p2.tile([128, 512], BF16, tag="qcb", bufs=2, name="qcb")
                        nc.vector.tensor_mul(qc, pq, cos_sb[:, ss])
                        nc.tensor.matmul(pq, perm_sb, t2, start=True, stop=True)
                        nc.vector.tensor_add(qkt[ct][:, ss], qc, pq)

                    return [pa, pb, pc]

                ob_box = {}

                def p3_half(it, nh, tag="flex", copy_eng=None):
                    # One half of an output-projection token tile.  As a
                    # filler piece the next flex allocation is >=1 unit away,
                    # so the PSUM->SBUF copy never blocks the PE.
                    ts = slice(128 * it, 128 * (it + 1))
                    ns = slice(512 * nh, 512 * (nh + 1))
                    if nh == 0:
                        ob_box[it] = p2.tile([128, C], BF16, tag="ob", bufs=4, name="ob")
                    ob = ob_box[it]
                    pp = p2ps.tile([128, 512], F32, tag=tag, bufs=1, name="pp3")
                    for p in range(2):
                        nc.tensor.matmul(
                            pp,
                            attn[p][:, ts],
                            wproj_sb[p][:, ns],
                            start=(p == 0),
                            stop=(p == 1),
                        )
                    if copy_eng is nc.scalar:
                        nc.scalar.copy(ob[:, ns], pp)
                    else:
                        nc.vector.tensor_copy(ob[:, ns], pp)
                    if nh == 1:
                        # Alternate queues to halve the drain pace; gpsimd is
                        # idle here (scalar would stall the exp stream).
                        eng = nc.sync if it % 2 == 0 else nc.gpsimd
                        eng.dma_start(out=out_d[ts, :], in_=ob_box.pop(it))

                def p3_pieces(s):
                    # Alternate PSUM tags so consecutive pieces ping-pong
                    # banks instead of waiting on each other's drain copy.
                    return [
                        (lambda it=it, nh=nh: p3_half(it, nh, tag=("flex", "rb")[nh]))
                        for it in range(4 * s, 4 * s + 4)
                        for nh in range(2)
                    ]

                pvs = {}
                pending = []

                def normalize(h, s):
                    # attn = pv[0:64] * (1/denom): broadcast the denominator
                    # row across partitions with a rank-1 fp32r matmul, invert
                    # on the DVE (no ACT involvement at all).
                    ct = h // 2
                    po = (h % 2) * 64
                    pv = pvs.pop((h, s))
                    dsb = p2.tile([65, 512], F32R, tag="dsb", bufs=2, name="dsb")
                    nc.vector.tensor_copy(dsb[64:65, :], pv[64:65, :])
                    rb = p2ps.tile([128, 512], F32, tag="rb", bufs=1, name="rb")
                    nc.tensor.matmul(
                        rb[0:64, :], ones_r[64:65, :], dsb[64:65, :],
                        start=True, stop=True,
                    )
                    rbf = p2.tile([64, 512], F32, tag="rbf", bufs=2, name="rbf")
                    nc.vector.reciprocal_approx_fast(rbf, rb[0:64, :])
                    nc.vector.tensor_mul(
                        attn[ct][po : po + 64, 512 * s : 512 * (s + 1)],
                        pv[0:64, :],
                        rbf,
                    )

                def emit_pv(item):
                    h, s, parts, et = item
                    if (h, s) not in pvs:
                        pvs[(h, s)] = p2ps.tile(
                            [65, 512], F32, tag="pv", bufs=2, name=f"pv{h}_{s}"
                        )
                    for (j, cstart, q0r, w) in parts:
                        nc.tensor.matmul(
                            pvs[(h, s)][:, q0r:512],
                            vaug[j][:, 65 * h : 65 * (h + 1)],
                            et[:, cstart : cstart + w],
                            start=(j == 0),
                            stop=(j == 4 * s + 3),
                        )
                        if j == 4 * s + 3:
                            normalize(h, s)

                def flush():
                    while pending:
                        emit_pv(pending.pop(0))

                def emit_unit(h, s, j0, j1):
                    ct = h // 2
                    po = (h % 2) * 64
                    qt, kt = qkt[ct], qkt[2 + ct]
                    sg = p2ps.tile([128, 1024], F32, tag="sg", bufs=2, name="sg")
                    parts = []
                    cstart = 0
                    for j in (j0, j1):
                        q0r = max(0, 128 * j - 512 * s)
                        w = 512 - q0r
                        if j != j0 and cstart + w > 512:
                            cstart = 512
                        diag = j // 4 == s
                        nc.tensor.matmul(
                            sg[:, cstart : cstart + w],
                            kt[po : po + 64, 128 * j : 128 * (j + 1)],
                            qt[po : po + 64, 512 * s + q0r : 512 * (s + 1)],
                            start=True,
                            stop=not diag,
                        )
                        if diag:
                            nc.tensor.matmul(
                                sg[:, cstart : cstart + 128],
                                mask_sb,
                                id_sb,
                                start=False,
                                stop=True,
                            )
                        parts.append((j, cstart, q0r, w))
                        cstart += w
                    et = p2.tile([128, 1024], BF16, tag="et", bufs=4, name="et")
                    nc.scalar.activation(
                        out=et[:, :cstart], in_=sg[:, :cstart],
                        func=EXP, scale=0.125,
                    )
                    pending.append((h, s, parts, et))
                    if len(pending) > 2:
                        emit_pv(pending.pop(0))

                def stream_pair(ha, hb, fillers=None, per_unit=1, span_hook=None):
                    # The two heads' units alternate so each span's exps and
                    # normalizes finish for BOTH heads together (this is what
                    # lets the output projection interleave into h2+h3).
                    if fillers is None:
                        fillers = []
                    delayed = []
                    for s in range(NSPAN):
                        if span_hook is not None:
                            new = span_hook(s)
                            if new:
                                # Activate only after two more units, by which
                                # point the pending lag has flushed the
                                # previous span's normalizes.
                                delayed.append([2, new])
                        js = list(range(4 * s + 4))
                        for u in range(0, len(js), 2):
                            for h in (ha, hb):
                                emit_unit(h, s, js[u], js[u + 1])
                            for dd in delayed:
                                dd[0] -= 1
                            while delayed and delayed[0][0] <= 0:
                                fillers.extend(delayed.pop(0)[1])
                            for _ in range(per_unit):
                                if fillers:
                                    fillers.pop(0)()

                # All QK-B combos fill the h0+h1 stream; the h2+h3 stream is
                # filled by the output projection, which pair-interleaving
                # makes available span by span (both heads of the pair
                # normalize each span together).
                f01 = []
                for sp in range(NSPAN):
                    f01 += qkb_pieces(3, sp) + qkb_pieces(1, sp)
                stream_pair(0, 1, fillers=f01)
                while f01:
                    f01.pop(0)()

                def h23_hook(s):
                    if s >= 1:
                        return p3_pieces(s - 1)
                    return None

                f23 = []
                stream_pair(2, 3, fillers=f23, per_unit=2, span_hook=h23_hook)
                flush()
                while f23:
                    f23.pop(0)()
                # Tail: the exps are done, so ACT is free — alternate PSUM
                # tags and copy engines so the PE never waits on a drain.
                copiers = [nc.vector, nc.scalar]
                for i, (it, nh) in enumerate(
                    [(it, nh) for it in range(12, NT) for nh in range(2)]
                ):
                    p3_half(it, nh, tag=("flex", "rb")[i % 2], copy_eng=copiers[i % 2])

    nc.compile()
    return nc


_NC = None


def _get_nc():
    global _NC
    if _NC is None:
        _NC = _build()
    return _NC


def _rope_tables():
    theta = (10000.0 ** (-np.arange(0, DH, 2, dtype=np.float32) / DH)).astype(
        np.float32
    )
    t = np.arange(T, dtype=np.float32)
    sinusoid = np.outer(t, theta).astype(np.float32)  # [T, DH/2]
    sin = np.concatenate([np.sin(sinusoid), np.sin(sinusoid)], axis=1)  # [T, DH]
    cos = np.concatenate([np.cos(sinusoid), np.cos(sinusoid)], axis=1)
    cosT = cos.T  # [DH, T]
    sinT = sin.T
    # sin_perm[e] = sin[(e+32) % 64]
    idx = (np.arange(DH) + 32) % DH
    sinTp = sinT[idx]
    cos2 = np.ascontiguousarray(np.concatenate([cosT, cosT], axis=0))  # [128, T]
    sinp2 = np.ascontiguousarray(np.concatenate([sinTp, sinTp], axis=0))
    return _bf(cos2), _bf(sinp2)


def _perm_matrix():
    p = np.zeros((128, 128), dtype=np.float32)
    for m in range(128):
        blk = m // 64
        k = blk * 64 + (m % 64 + 32) % 64
        p[k, m] = 1.0
    return p


def _mask_matrices():
    # maskT.T @ I adds -400 to S^T[k, q] where k > q (then exp(0.125*s)=0):
    # maskT[a, b] = -400 where b > a
    maskT = -400.0 * np.triu(np.ones((128, 128), dtype=np.float32), 1)
    return maskT, np.eye(128, dtype=np.float32)


def _bf(a):
    return np.ascontiguousarray(np.asarray(a, dtype=np.float32).astype(NPBF16))


def _prepare_in_maps(x, w_qkv, b_qkv, w_proj):
    x = np.asarray(x, dtype=np.float32)
    w_qkv = np.asarray(w_qkv, dtype=np.float32)
    b_qkv = np.asarray(b_qkv, dtype=np.float32)
    w_proj = np.asarray(w_proj, dtype=np.float32)

    cos2, sinp2 = _rope_tables()
    perm = _bf(_perm_matrix())
    maskT, id128 = _mask_matrices()
    maskT, id128 = _bf(maskT), _bf(id128)
    xTs = [_bf(x[b].T) for b in range(B)]

    in_maps = []
    for c in range(N_CORES):
        b, g = divmod(c, 4)
        h0 = g * GH  # first head of the group
        qcols = w_qkv[:, h0 * DH : (h0 + GH) * DH]
        kcols = w_qkv[:, C + h0 * DH : C + (h0 + GH) * DH]
        wqk = _bf(np.concatenate([qcols, kcols], axis=1))
        wv = np.zeros((C, VA), dtype=np.float32)
        bv = np.zeros((1, VA), dtype=np.float32)
        for j in range(GH):
            src = 2 * C + (h0 + j) * DH
            wv[:, j * 65 : j * 65 + DH] = w_qkv[:, src : src + DH]
            bv[0, j * 65 : j * 65 + DH] = b_qkv[src : src + DH]
            bv[0, j * 65 + DH] = 1.0
        bqk = np.concatenate(
            [b_qkv[h0 * DH : (h0 + GH) * DH], b_qkv[C + h0 * DH : C + (h0 + GH) * DH]]
        ).reshape(1, QK_COLS)
        wproj = np.stack(
            [w_proj[(h0 + 2 * p) * DH : (h0 + 2 * p + 2) * DH, :] for p in range(2)]
        )
        in_maps.append(
            {
                "xT": xTs[b],
                "wqk": wqk,
                "wv": _bf(wv),
                "bqk": _bf(bqk),
                "bv": _bf(bv),
                "cosT": cos2,
                "sinTp": sinp2,
                "perm": perm,
                "maskT": maskT,
                "id128": id128,
                "wproj": _bf(wproj),
            }
        )
    return in_maps


def run(x, w_qkv, b_qkv, w_proj, b_proj, trace=False, tmpdir=None):
    nc = _get_nc()
    in_maps = _prepare_in_maps(x, w_qkv, b_qkv, w_proj)
    # The very first execution of a freshly compiled NEFF occasionally hits a
    # transient NRT_EXEC_UNIT_UNRECOVERABLE; a retry on a fresh session has
    # always succeeded.
    try:
        res = run_bass_kernel_spmd(
            nc, in_maps, list(range(N_CORES)), trace=trace, tmpdir=tmpdir
        )
    except Exception:
        res = run_bass_kernel_spmd(
            nc, in_maps, list(range(N_CORES)), trace=trace, tmpdir=tmpdir
        )
    b_proj = np.asarray(b_proj, dtype=np.float32)
    out = np.empty((B, T, C), dtype=np.float32)
    for b in range(B):
        acc = res.results[4 * b]["out"].astype(np.float32)
        for g in range(1, 4):
            acc = acc + res.results[4 * b + g]["out"].astype(np.float32)
        out[b] = acc + b_proj
    return out, res


def kernel(x, w_qkv, b_qkv, w_proj, b_proj):
    out, _ = run(x, w_qkv, b_qkv, w_proj, b_proj, trace=False)
    return out


# revision 35
# speedup vs baseline: 1.0302x; 1.0302x over previous
"""Multi-head causal attention with RoPE on 8 Trainium2 NeuronCores.

Reference computation (B=2, T=2048, C=1024, H=16, Dh=64, fp32):
    qkv = x @ w_qkv + b_qkv ; split q,k,v ; RoPE(q), RoPE(k)
    attn = softmax_causal(q k^T / sqrt(Dh)) @ v ; out = attn @ w_proj + b_proj

Sharding: core c = b*4 + g handles batch b and head group g (heads 4g..4g+3).
Data-parallel over batch, tensor-parallel over heads (w_qkv column-split,
w_proj row-split).  Each core emits a partial [T, C] projection output in
bf16 (halves the write drain); the host sums the 4 per-batch partials in
fp32 and adds b_proj.

Per-core kernel, organized to keep the PE tensor engine saturated end to end
(the HAM clock gate re-throttles the PE to 1.2 GHz after any idle window, so
PE gaps are doubly expensive):
  - DMA: x^T chunks stream on the sync+scalar queues while weights lead the
    gpsimd queue; rope tables (bf16) and w_proj ride the vector queue.  The
    first QKV matmuls start as soon as chunk 0 lands.
  - Phase 1a computes Q^T/K^T for heads 0-1 CHUNK-major across 8 PSUM banks
    so the PE chases the x DMA instead of stalling on the full tensor.
    RoPE fuses bias into the accumulation (rank-1 matmul), then
    qkt = pq*cos + perm @ (pq*sin_perm), the permutation matmul reusing the
    same PSUM bank in place.
  - Phase 1b computes V in [token, head*65] layout (65th col = ones so the
    PV matmul also emits the softmax denominator).
  - Phase 2 streams attention per head, span by span, in units of TWO
    k-tiles sharing a [128, 1024] two-bank PSUM group: one exp ACTIVATE
    covers both tiles (the ACT engine has a 352-cycle fixed cost per
    instruction, so batching matters), with the 1/sqrt(Dh) scale fused and
    causality via per-tile q-range narrowing plus a triangular mask matmul
    on diagonal tiles.  S^T tiles (scores transposed) make the softmax sum
    direction match the PE contraction.  A 2-unit lookahead queue keeps the
    PE 2 units ahead of the ACT exp.
  - Softmax normalization has NO ACT work (the baseline's exp(-ln(d)) tables
    thrashed ACT_TABLE_LOAD): the denominator row is broadcast across
    partitions with a rank-1 fp32r matmul, inverted with the DVE
    reciprocal_approx_fast op (~18 bits, plenty for bf16 storage), and
    multiplied in on the DVE.
  - Q/K for heads 2-3 are computed BETWEEN the head 0-1 and head 2-3
    attention streams (kc-inner, one flex PSUM bank), and the output
    projection for spans 0-2 is interleaved INTO head 3's stream so the
    [T, C] fp32 result DMA overlaps compute instead of trailing it.
All heavy matmuls run in bf16 (fp32 accumulation in PSUM); end-to-end rel
err ~5e-3 of output absmax.
"""

import numpy as np
import ml_dtypes

import concourse.bacc as bacc
import concourse.bass as bass
import concourse.mybir as mybir
from concourse.tile import TileContext
from concourse.bass_utils import run_bass_kernel_spmd

F32 = mybir.dt.float32
F32R = mybir.dt.float32r
BF16 = mybir.dt.bfloat16
NPBF16 = np.dtype(ml_dtypes.bfloat16)

B, T, C = 2, 2048, 1024
H, DH = 16, 64
GH = 4  # heads per core
N_CORES = 8
NCHUNK = C // 128  # 8 contraction chunks
NT = T // 128  # 16 token tiles
NSPAN = T // 512  # 4 query spans
QK_COLS = 2 * GH * DH  # 512 = q cols (256) + k cols (256)
VA = GH * (DH + 1)  # 260 = v cols augmented with ones column per head
EXP = mybir.ActivationFunctionType.Exp


def _build():
    nc = bacc.Bacc("TRN2", target_bir_lowering=False, debug=False, num_devices=N_CORES)

    xT = nc.dram_tensor("xT", [C, T], BF16, kind="ExternalInput")
    wqk = nc.dram_tensor("wqk", [C, QK_COLS], BF16, kind="ExternalInput")
    wv = nc.dram_tensor("wv", [C, VA], BF16, kind="ExternalInput")
    bqk_d = nc.dram_tensor("bqk", [1, QK_COLS], BF16, kind="ExternalInput")
    bv_d = nc.dram_tensor("bv", [1, VA], BF16, kind="ExternalInput")
    cos_d = nc.dram_tensor("cosT", [128, T], BF16, kind="ExternalInput")
    sinp_d = nc.dram_tensor("sinTp", [128, T], BF16, kind="ExternalInput")
    perm_d = nc.dram_tensor("perm", [128, 128], BF16, kind="ExternalInput")
    maskT_d = nc.dram_tensor("maskT", [128, 128], BF16, kind="ExternalInput")
    id_d = nc.dram_tensor("id128", [128, 128], BF16, kind="ExternalInput")
    wproj_d = nc.dram_tensor("wproj", [2, 128, C], BF16, kind="ExternalInput")
    # bf16 output halves the 8 MiB write drain; the host sums the four
    # per-batch partials in fp32.
    out_d = nc.dram_tensor("out", [T, C], BF16, kind="ExternalOutput")

    with TileContext(nc) as tc:
        with tc.tile_pool(name="persist", bufs=1) as pers:
            # x^T chunks lead the two hardware DMA queues (they gate the
            # phase-1a chase); wqk leads the gpsimd software queue (its ~12us
            # startup latency roughly matches the first x chunks' arrival).
            xt = []
            for kc in range(NCHUNK):
                t = pers.tile([128, T], BF16, tag="xt", bufs=NCHUNK, name=f"xt{kc}")
                eng = nc.scalar if kc % 2 == 0 else nc.sync
                eng.dma_start(out=t, in_=xT[128 * kc : 128 * (kc + 1), :])
                xt.append(t)
            wqk_t = []
            for kc in range(NCHUNK):
                t = pers.tile(
                    [128, QK_COLS], BF16, tag="wqk", bufs=NCHUNK, name=f"wqk{kc}"
                )
                nc.gpsimd.dma_start(out=t, in_=wqk[128 * kc : 128 * (kc + 1), :])
                wqk_t.append(t)
            cos_sb = pers.tile([128, T], BF16, tag="cos")
            nc.gpsimd.dma_start(out=cos_sb, in_=cos_d[:, :])
            sinp_sb = pers.tile([128, T], BF16, tag="sinp")
            nc.gpsimd.dma_start(out=sinp_sb, in_=sinp_d[:, :])
            wv_t = []
            for kc in range(NCHUNK):
                t = pers.tile([128, VA], BF16, tag="wv", bufs=NCHUNK, name=f"wv{kc}")
                nc.gpsimd.dma_start(out=t, in_=wv[128 * kc : 128 * (kc + 1), :])
                wv_t.append(t)
            bqk_sb = pers.tile([1, QK_COLS], BF16, tag="bqk")
            nc.gpsimd.dma_start(out=bqk_sb, in_=bqk_d[:, :])
            bv_sb = pers.tile([1, VA], BF16, tag="bv")
            nc.gpsimd.dma_start(out=bv_sb, in_=bv_d[:, :])
            perm_sb = pers.tile([128, 128], BF16, tag="perm")
            nc.gpsimd.dma_start(out=perm_sb, in_=perm_d[:, :])
            mask_sb = pers.tile([128, 128], BF16, tag="maskT")
            nc.gpsimd.dma_start(out=mask_sb, in_=maskT_d[:, :])
            id_sb = pers.tile([128, 128], BF16, tag="id128")
            nc.gpsimd.dma_start(out=id_sb, in_=id_d[:, :])
            wproj_sb = []
            for p in range(2):
                t = pers.tile([128, C], BF16, tag="wproj", bufs=2, name=f"wproj{p}")
                nc.gpsimd.dma_start(out=t, in_=wproj_d[p, :, :])
                wproj_sb.append(t)

            ones = pers.tile([1, 512], BF16, tag="ones")
            nc.vector.memset(ones, 1.0)
            ones_ff = pers.tile([128, 64], F32, tag="ones_ff")
            nc.vector.memset(ones_ff, 1.0)
            ones_r = pers.tile([128, 64], F32R, tag="ones_r")
            nc.vector.tensor_copy(ones_r, ones_ff)

            # Outputs of phase 1 (live into phase 2/3)
            qkt = []  # 4 tiles [128, T]: Q heads(0,1), Q(2,3), K(0,1), K(2,3)
            for i in range(4):
                t = pers.tile([128, T], BF16, tag="qkt", bufs=4, name=f"qkt{i}")
                qkt.append(t)
            vaug = []  # 16 tiles [128, VA], k-tile-major natural layout V
            for j in range(NT):
                t = pers.tile([128, VA], BF16, tag="vaug", bufs=NT, name=f"vaug{j}")
                vaug.append(t)
            attn = []  # 2 tiles [128, T]: normalized attn^T for head pairs
            for p in range(2):
                t = pers.tile([128, T], BF16, tag="attn", bufs=2, name=f"attn{p}")
                attn.append(t)

            # ------------- Phase 1a: Q/K heads 0-1, chunk-major -------------
            # One PSUM bank per (ct, span); all 8 banks live so each arriving
            # x chunk feeds 8 back-to-back matmuls.  ct=2 (K) first so the
            # attention stream's S matmuls unblock earliest.  After the bias,
            # each combo is staged to SBUF bf16 by the idle ACT engine: this
            # frees the PSUM banks immediately so V starts right away, and
            # the rope finish runs as bf16 DVE work overlapped with V.
            combosA = [(ct, sp) for ct in (2, 0) for sp in range(NSPAN)]
            with tc.tile_pool(name="p1a", bufs=1) as p1a:
                pqsb = []
                with tc.tile_pool(name="p1aps", bufs=1, space="PSUM") as p1aps:
                    psqk = [
                        p1aps.tile([128, 512], F32, tag="psqk", bufs=8, name=f"psqk{i}")
                        for i in range(8)
                    ]
                    for kc in range(NCHUNK):
                        for i, (ct, sp) in enumerate(combosA):
                            cs = slice(128 * ct, 128 * (ct + 1))
                            ss = slice(512 * sp, 512 * (sp + 1))
                            nc.tensor.matmul(
                                psqk[i],
                                wqk_t[kc][:, cs],
                                xt[kc][:, ss],
                                start=(kc == 0),
                                stop=False,
                            )
                    # Staging order matters: phase 1b's psv tiles reuse the
                    # banks of the first-staged combos (V chases copies 0-5 in
                    # order) while the rope's ppr tiles land on the last two
                    # banks — so combos 6,7 are staged right after 0,1 or the
                    # first perm matmul waits for the final copy.
                    pqsb = [None] * 8
                    for i in (0, 1, 6, 7, 2, 3, 4, 5):
                        ct, sp = combosA[i]
                        cs = slice(128 * ct, 128 * (ct + 1))
                        nc.tensor.matmul(
                            psqk[i], bqk_sb[0:1, cs], ones, start=False, stop=True
                        )
                        t = p1a.tile([128, 512], BF16, tag="pqsb", bufs=8, name="pqsb")
                        # All staging copies on ACT: psv bufs=6 gives the V
                        # chain enough slack that queueing behind these is
                        # fine, and the DVE stays clear for the rope chain.
                        nc.scalar.copy(t, psqk[i])
                        pqsb[i] = t

                # ------------- Phase 1b: V + rope finish --------------------
                # V's vaug copies ride ACT; the rope combos (2 bf16 DVE muls,
                # one perm matmul, one DVE add) are injected between V tiles
                # so the DVE rope chain hides entirely under V's PE work.
                with tc.tile_pool(name="p1bps", bufs=1, space="PSUM") as p1bps:
                    def rope_finish(i):
                        ct, sp = combosA[i]
                        ss = slice(512 * sp, 512 * (sp + 1))
                        t2 = p1a.tile([128, 512], BF16, tag="t2", bufs=2, name="t2")
                        nc.vector.tensor_mul(t2, pqsb[i], sinp_sb[:, ss])
                        qc = p1a.tile([128, 512], BF16, tag="qc", bufs=2, name="qc")
                        nc.vector.tensor_mul(qc, pqsb[i], cos_sb[:, ss])
                        pp = p1bps.tile(
                            [128, 512], F32, tag="ppr", bufs=2, name="ppr"
                        )
                        nc.tensor.matmul(pp, perm_sb, t2, start=True, stop=True)
                        nc.vector.tensor_add(qkt[ct][:, ss], qc, pp)

                    for it in range(NT):
                        pv = p1bps.tile([128, VA], F32, tag="psv", bufs=6, name="psv")
                        ts = slice(128 * it, 128 * (it + 1))
                        for kc in range(NCHUNK):
                            nc.tensor.matmul(
                                pv, xt[kc][:, ts], wv_t[kc],
                                start=(kc == 0), stop=False,
                            )
                        nc.tensor.matmul(
                            pv, ones[0:1, 0:128], bv_sb, start=False, stop=True
                        )
                        nc.scalar.copy(vaug[it], pv)
                        # Tiles 0-3 run clean while ACT finishes the pqsb
                        # staging; the rope chain starts at tile 4 with its
                        # inputs already in SBUF.
                        if 4 <= it < 12:
                            rope_finish(it - 4)

            # ------------- Phase 2: attention + interleaved extras ----------
            # Q/K for heads 2-3 and the output projection run as PE filler
            # INSIDE the attention streams: the per-unit exp on ACT (1.15us)
            # outpaces the unit's own matmuls (0.85us), and any sustained PE
            # idle re-throttles the HAM clock gate to 1.2 GHz.
            with (
                tc.tile_pool(name="p2", bufs=1) as p2,
                tc.tile_pool(name="p2ps", bufs=1, space="PSUM") as p2ps,
            ):

                def qkb_pieces(ct, sp):
                    # Q/K heads 2-3, one span column: split into 3 filler
                    # pieces (the flex PSUM bank is held across all three).
                    cs = slice(128 * ct, 128 * (ct + 1))
                    ss = slice(512 * sp, 512 * (sp + 1))
                    box = {}

                    def pa():
                        pq = p2ps.tile(
                            [128, 512], F32, tag="flex", bufs=1, name="pqB"
                        )
                        box["pq"] = pq
                        for kc in range(4):
                            nc.tensor.matmul(
                                pq, wqk_t[kc][:, cs], xt[kc][:, ss],
                                start=(kc == 0), stop=False,
                            )

                    def pb():
                        pq = box["pq"]
                        for kc in range(4, NCHUNK):
                            nc.tensor.matmul(
                                pq, wqk_t[kc][:, cs], xt[kc][:, ss],
                                start=False, stop=False,
                            )
                        nc.tensor.matmul(
                            pq, bqk_sb[0:1, cs], ones, start=False, stop=True
                        )

                    def pc():
                        pq = box["pq"]
                        t2 = p2.tile([128, 512], BF16, tag="t2b", bufs=2, name="t2b")
                        nc.vector.tensor_mul(t2, pq, sinp_sb[:, ss])
                        qc = p2.tile([128, 512], BF16, tag="qcb", bufs=2, name="qcb")
                        nc.vector.tensor_mul(qc, pq, cos_sb[:, ss])
                        nc.tensor.matmul(pq, perm_sb, t2, start=True, stop=True)
                        nc.vector.tensor_add(qkt[ct][:, ss], qc, pq)

                    return [pa, pb, pc]

                ob_box = {}

                def p3_half(it, nh, tag="flex", copy_eng=None):
                    # One half of an output-projection token tile.  As a
                    # filler piece the next flex allocation is >=1 unit away,
                    # so the PSUM->SBUF copy never blocks the PE.
                    ts = slice(128 * it, 128 * (it + 1))
                    ns = slice(512 * nh, 512 * (nh + 1))
                    if nh == 0:
                        ob_box[it] = p2.tile([128, C], BF16, tag="ob", bufs=4, name="ob")
                    ob = ob_box[it]
                    pp = p2ps.tile([128, 512], F32, tag=tag, bufs=1, name="pp3")
                    for p in range(2):
                        nc.tensor.matmul(
                            pp,
                            attn[p][:, ts],
                            wproj_sb[p][:, ns],
                            start=(p == 0),
                            stop=(p == 1),
                        )
                    if copy_eng is nc.scalar:
                        nc.scalar.copy(ob[:, ns], pp)
                    else:
                        nc.vector.tensor_copy(ob[:, ns], pp)
                    if nh == 1:
                        # Alternate queues to halve the drain pace; gpsimd is
                        # idle here (scalar would stall the exp stream).
                        eng = nc.sync if it % 2 == 0 else nc.gpsimd
                        eng.dma_start(out=out_d[ts, :], in_=ob_box.pop(it))

                def p3_pieces(s):
                    # Alternate PSUM tags so consecutive pieces ping-pong
                    # banks instead of waiting on each other's drain copy.
                    return [
                        (lambda it=it, nh=nh: p3_half(it, nh, tag=("flex", "rb")[nh]))
                        for it in range(4 * s, 4 * s + 4)
                        for nh in range(2)
                    ]

                pvs = {}
                pending = []

                def normalize(h, s):
                    # attn = pv[0:64] * (1/denom): broadcast the denominator
                    # row across partitions with a rank-1 fp32r matmul, invert
                    # on the DVE (no ACT involvement at all).
                    ct = h // 2
                    po = (h % 2) * 64
                    pv = pvs.pop((h, s))
                    dsb = p2.tile([65, 512], F32R, tag="dsb", bufs=2, name="dsb")
                    nc.vector.tensor_copy(dsb[64:65, :], pv[64:65, :])
                    rb = p2ps.tile([128, 512], F32, tag="rb", bufs=1, name="rb")
                    nc.tensor.matmul(
                        rb[0:64, :], ones_r[64:65, :], dsb[64:65, :],
                        start=True, stop=True,
                    )
                    rbf = p2.tile([64, 512], F32, tag="rbf", bufs=2, name="rbf")
                    nc.vector.reciprocal_approx_fast(rbf, rb[0:64, :])
                    nc.vector.tensor_mul(
                        attn[ct][po : po + 64, 512 * s : 512 * (s + 1)],
                        pv[0:64, :],
                        rbf,
                    )

                def emit_pv(item):
                    h, s, parts, et = item
                    if (h, s) not in pvs:
                        pvs[(h, s)] = p2ps.tile(
                            [65, 512], F32, tag="pv", bufs=2, name=f"pv{h}_{s}"
                        )
                    for (j, cstart, q0r, w) in parts:
                        nc.tensor.matmul(
                            pvs[(h, s)][:, q0r:512],
                            vaug[j][:, 65 * h : 65 * (h + 1)],
                            et[:, cstart : cstart + w],
                            start=(j == 0),
                            stop=(j == 4 * s + 3),
                        )
                        if j == 4 * s + 3:
                            normalize(h, s)

                def flush():
                    while pending:
                        emit_pv(pending.pop(0))

                def emit_unit(h, s, j0, j1):
                    ct = h // 2
                    po = (h % 2) * 64
                    qt, kt = qkt[ct], qkt[2 + ct]
                    sg = p2ps.tile([128, 1024], F32, tag="sg", bufs=2, name="sg")
                    parts = []
                    cstart = 0
                    for j in (j0, j1):
                        q0r = max(0, 128 * j - 512 * s)
                        w = 512 - q0r
                        if j != j0 and cstart + w > 512:
                            cstart = 512
                        diag = j // 4 == s
                        nc.tensor.matmul(
                            sg[:, cstart : cstart + w],
                            kt[po : po + 64, 128 * j : 128 * (j + 1)],
                            qt[po : po + 64, 512 * s + q0r : 512 * (s + 1)],
                            start=True,
                            stop=not diag,
                        )
                        if diag:
                            nc.tensor.matmul(
                                sg[:, cstart : cstart + 128],
                                mask_sb,
                                id_sb,
                                start=False,
                                stop=True,
                            )
                        parts.append((j, cstart, q0r, w))
                        cstart += w
                    et = p2.tile([128, 1024], BF16, tag="et", bufs=5, name="et")
                    nc.scalar.activation(
                        out=et[:, :cstart], in_=sg[:, :cstart],
                        func=EXP, scale=0.125,
                    )
                    pending.append((h, s, parts, et))
                    # Depth 3: one extra unit of ACT slack before the PE
                    # needs the exp result for the PV matmuls.
                    if len(pending) > 3:
                        emit_pv(pending.pop(0))

                def stream_pair(ha, hb, fillers=None, per_unit=1, span_hook=None):
                    # The two heads' units alternate so each span's exps and
                    # normalizes finish for BOTH heads together (this is what
                    # lets the output projection interleave into h2+h3).
                    if fillers is None:
                        fillers = []
                    delayed = []
                    for s in range(NSPAN):
                        if span_hook is not None:
                            new = span_hook(s)
                            if new:
                                # Activate only after two more units, by which
                                # point the pending lag has flushed the
                                # previous span's normalizes.
                                delayed.append([2, new])
                        js = list(range(4 * s + 4))
                        for u in range(0, len(js), 2):
                            for h in (ha, hb):
                                emit_unit(h, s, js[u], js[u + 1])
                            for dd in delayed:
                                dd[0] -= 1
                            while delayed and delayed[0][0] <= 0:
                                fillers.extend(delayed.pop(0)[1])
                            for _ in range(per_unit):
                                if fillers:
                                    fillers.pop(0)()

                # All QK-B combos fill the h0+h1 stream; the h2+h3 stream is
                # filled by the output projection, which pair-interleaving
                # makes available span by span (both heads of the pair
                # normalize each span together).
                f01 = []
                for sp in range(NSPAN):
                    f01 += qkb_pieces(3, sp) + qkb_pieces(1, sp)
                stream_pair(0, 1, fillers=f01)
                while f01:
                    f01.pop(0)()

                def h23_hook(s):
                    if s >= 1:
                        return p3_pieces(s - 1)
                    return None

                f23 = []
                stream_pair(2, 3, fillers=f23, per_unit=2, span_hook=h23_hook)
                flush()
                while f23:
                    f23.pop(0)()
                # Tail: the exps are done, so ACT is free — alternate PSUM
                # tags and copy engines so the PE never waits on a drain.
                copiers = [nc.vector, nc.scalar]
                for i, (it, nh) in enumerate(
                    [(it, nh) for it in range(12, NT) for nh in range(2)]
                ):
                    p3_half(it, nh, tag=("flex", "rb")[i % 2], copy_eng=copiers[i % 2])

    nc.compile()
    return nc


_NC = None


def _get_nc():
    global _NC
    if _NC is None:
        _NC = _build()
    return _NC


def _rope_tables():
    theta = (10000.0 ** (-np.arange(0, DH, 2, dtype=np.float32) / DH)).astype(
        np.float32
    )
    t = np.arange(T, dtype=np.float32)
    sinusoid = np.outer(t, theta).astype(np.float32)  # [T, DH/2]
    sin = np.concatenate([np.sin(sinusoid), np.sin(sinusoid)], axis=1)  # [T, DH]
    cos = np.concatenate([np.cos(sinusoid), np.cos(sinusoid)], axis=1)
    cosT = cos.T  # [DH, T]
    sinT = sin.T
    # sin_perm[e] = sin[(e+32) % 64]
    idx = (np.arange(DH) + 32) % DH
    sinTp = sinT[idx]
    cos2 = np.ascontiguousarray(np.concatenate([cosT, cosT], axis=0))  # [128, T]
    sinp2 = np.ascontiguousarray(np.concatenate([sinTp, sinTp], axis=0))
    return _bf(cos2), _bf(sinp2)


def _perm_matrix():
    p = np.zeros((128, 128), dtype=np.float32)
    for m in range(128):
        blk = m // 64
        k = blk * 64 + (m % 64 + 32) % 64
        p[k, m] = 1.0
    return p


def _mask_matrices():
    # maskT.T @ I adds -400 to S^T[k, q] where k > q (then exp(0.125*s)=0):
    # maskT[a, b] = -400 where b > a
    maskT = -400.0 * np.triu(np.ones((128, 128), dtype=np.float32), 1)
    return maskT, np.eye(128, dtype=np.float32)


def _bf(a):
    return np.ascontiguousarray(np.asarray(a, dtype=np.float32).astype(NPBF16))


def _prepare_in_maps(x, w_qkv, b_qkv, w_proj):
    x = np.asarray(x, dtype=np.float32)
    w_qkv = np.asarray(w_qkv, dtype=np.float32)
    b_qkv = np.asarray(b_qkv, dtype=np.float32)
    w_proj = np.asarray(w_proj, dtype=np.float32)

    cos2, sinp2 = _rope_tables()
    perm = _bf(_perm_matrix())
    maskT, id128 = _mask_matrices()
    maskT, id128 = _bf(maskT), _bf(id128)
    xTs = [_bf(x[b].T) for b in range(B)]

    in_maps = []
    for c in range(N_CORES):
        b, g = divmod(c, 4)
        h0 = g * GH  # first head of the group
        qcols = w_qkv[:, h0 * DH : (h0 + GH) * DH]
        kcols = w_qkv[:, C + h0 * DH : C + (h0 + GH) * DH]
        wqk = _bf(np.concatenate([qcols, kcols], axis=1))
        wv = np.zeros((C, VA), dtype=np.float32)
        bv = np.zeros((1, VA), dtype=np.float32)
        for j in range(GH):
            src = 2 * C + (h0 + j) * DH
            wv[:, j * 65 : j * 65 + DH] = w_qkv[:, src : src + DH]
            bv[0, j * 65 : j * 65 + DH] = b_qkv[src : src + DH]
            bv[0, j * 65 + DH] = 1.0
        bqk = np.concatenate(
            [b_qkv[h0 * DH : (h0 + GH) * DH], b_qkv[C + h0 * DH : C + (h0 + GH) * DH]]
        ).reshape(1, QK_COLS)
        wproj = np.stack(
            [w_proj[(h0 + 2 * p) * DH : (h0 + 2 * p + 2) * DH, :] for p in range(2)]
        )
        in_maps.append(
            {
                "xT": xTs[b],
                "wqk": wqk,
                "wv": _bf(wv),
                "bqk": _bf(bqk),
                "bv": _bf(bv),
                "cosT": cos2,
                "sinTp": sinp2,
                "perm": perm,
                "maskT": maskT,
                "id128": id128,
                "wproj": _bf(wproj),
            }
        )
    return in_maps


def run(x, w_qkv, b_qkv, w_proj, b_proj, trace=False, tmpdir=None):
    nc = _get_nc()
    in_maps = _prepare_in_maps(x, w_qkv, b_qkv, w_proj)
    # The very first execution of a freshly compiled NEFF occasionally hits a
    # transient NRT_EXEC_UNIT_UNRECOVERABLE; a retry on a fresh session has
    # always succeeded.
    try:
        res = run_bass_kernel_spmd(
            nc, in_maps, list(range(N_CORES)), trace=trace, tmpdir=tmpdir
        )
    except Exception:
        res = run_bass_kernel_spmd(
            nc, in_maps, list(range(N_CORES)), trace=trace, tmpdir=tmpdir
        )
    b_proj = np.asarray(b_proj, dtype=np.float32)
    out = np.empty((B, T, C), dtype=np.float32)
    for b in range(B):
        acc = res.results[4 * b]["out"].astype(np.float32)
        for g in range(1, 4):
            acc = acc + res.results[4 * b + g]["out"].astype(np.float32)
        out[b] = acc + b_proj
    return out, res


def kernel(x, w_qkv, b_qkv, w_proj, b_proj):
    out, _ = run(x, w_qkv, b_qkv, w_proj, b_proj, trace=False)
    return out
